# revision 1
# baseline (speedup 1.0000x reference)
"""AGNN propagation kernel for 8 TRN2 NeuronCores.

Algorithm (matches reference):
    x_norm = x * rsqrt(sum(x^2, -1) + 1e-8)
    logit_e = beta * <x_norm[dst_e], x_norm[src_e]>        (in [-beta, beta])
    alpha_e = exp(logit_e) / (segsum_dst(exp(logit)) + 1e-8)
    out_i   = sum_{e: dst_e = i} alpha_e * x[src_e]

Because |logit| <= beta < 1, the segment-max subtraction in the reference is
numerically unnecessary (exp stays in [e^-1, e]); plain exp matches to ~1e-7.

Sharding: node-parallel, no collectives. Host sorts nodes by in-degree and
stripes them across the 8 cores (rank c, c+8, ...), so every core sees an
identical degree profile. Each core packs its nodes into blocks of 128
(1 node per SBUF partition); block b is padded to K_b = max degree in the
block (tight, because nodes are degree-sorted). A single indirect DMA per
block-group gathers the packed [x_norm | ||x||] rows of every edge source
into [128 nodes, K, 36] tiles; the segment softmax + weighted aggregation
are then per-partition free-dim reduces. Pad edge slots point at an
all-zero dummy row and are masked out of the softmax denominator.

Device phases (per core, one SPMD graph):
  prep: stream x -> packed[n] = [x_norm(32) | sqrt(ss+eps)(1) | pad(3)]
  main: per block: indirect-gather src rows, DVE dot products vs the
        block's own x_norm rows (per-partition broadcast), ScalarE exp,
        mask pads, free-dim reduces, divide, write dense output rows.
Host reassembles: out[node_order] = dense rows.
"""

import os
import numpy as np

import concourse.bass as bass
import concourse.bacc as bacc
import concourse.mybir as mybir
import concourse.tile as tile

F32 = mybir.dt.float32
I32 = mybir.dt.int32

N_CORES = 8
LAST_RESULT = None  # set by kernel() for profiling harnesses
P = 128          # SBUF partitions (= nodes per block)
EPS = 1e-8
PREP_ROWS = 16   # node rows per partition per prep supertile
BLOCKS_PER_GATHER = 2


# ----------------------------------------------------------------------------
# Host-side planning (index manipulation only; no FLOPs on tensor data)
# ----------------------------------------------------------------------------

def build_plan(edge_index: np.ndarray, n_nodes: int):
    src = np.asarray(edge_index[0], dtype=np.int64)
    dst = np.asarray(edge_index[1], dtype=np.int64)
    n_edges = src.shape[0]

    deg = np.bincount(dst, minlength=n_nodes).astype(np.int64)

    # Global degree-descending node order, striped over cores.
    order = np.argsort(-deg, kind="stable")

    nodes_per_core = (n_nodes + N_CORES - 1) // N_CORES
    blocks = (nodes_per_core + P - 1) // P
    slots = blocks * P                      # padded nodes per core

    # CSR of incoming edges (sorted by dst).
    eorder = np.argsort(dst, kind="stable")
    src_sorted = src[eorder]
    starts = np.zeros(n_nodes + 1, dtype=np.int64)
    np.cumsum(deg, out=starts[1:])

    # Shared block K profile: block b holds global ranks [b*P*NC, (b+1)*P*NC).
    deg_ranked = deg[order]
    K = np.zeros(blocks, dtype=np.int64)
    for b in range(blocks):
        lo = b * P * N_CORES
        hi = min(lo + P * N_CORES, n_nodes)
        K[b] = max(1, int(deg_ranked[lo:hi].max()) if hi > lo else 1)
    offs = np.zeros(blocks + 1, dtype=np.int64)
    np.cumsum(K, out=offs[1:])
    totk = int(offs[-1])

    dummy = n_nodes  # row of zeros in the padded x

    # node_of[c, s]: global node id at core c, slot s (or -1 pad).
    ranks = np.arange(slots) * N_CORES  # slot -> global rank base
    node_of = np.full((N_CORES, slots), -1, dtype=np.int64)
    for c in range(N_CORES):
        r = ranks + c
        valid = r < n_nodes
        node_of[c, valid] = order[r[valid]]

    return dict(
        n_nodes=n_nodes, n_edges=n_edges, blocks=blocks, slots=slots,
        K=K, offs=offs, totk=totk, dummy=dummy, node_of=node_of,
    )


# ----------------------------------------------------------------------------
# Bass graph builder (one SPMD graph shared by all cores)
# ----------------------------------------------------------------------------

def build_kernel(n_nodes_pad: int, d_feat: int, blocks: int, K, offs, totk: int,
                 prep_rows: int = PREP_ROWS, b_mini: int = 0, mini_rows: int = 0):
    """n_nodes_pad: padded row count of x input (multiple of P*prep_rows).

    b_mini: first b_mini blocks gather from a small dedicated packed_mini
    table (host-deduped source rows, remapped indices) whose prep finishes
    ~5x sooner than the full table, hiding the prep head under gathers.
    """
    assert n_nodes_pad % (P * prep_rows) == 0
    supertiles = n_nodes_pad // (P * prep_rows)
    D = d_feat
    DP = D + 4   # packed row: [x_norm(D) | w(1) | pad(3)] -> 16B aligned for D=32
    kmax = int(max(K))
    assert kmax <= 512
    m_cols = int(offs[b_mini]) if b_mini else 0
    assert b_mini == 0 or mini_rows % (P * prep_rows) == 0

    nc = bacc.Bacc(None, target_bir_lowering=False, debug=False)

    x_ext = nc.declare_dram_parameter("x", [n_nodes_pad, D], F32, isOutput=False)
    idx_ext = nc.declare_dram_parameter("idx", [P, totk], I32, isOutput=False)
    xd_ext = nc.declare_dram_parameter("xd", [P, blocks * D], F32, isOutput=False)
    deg_ext = nc.declare_dram_parameter("deg", [P, blocks], F32, isOutput=False)
    beta_ext = nc.declare_dram_parameter("beta", [P, 1], F32, isOutput=False)
    if b_mini:
        xm_ext = nc.declare_dram_parameter("xm", [mini_rows, D], F32,
                                           isOutput=False)
        idxm_ext = nc.declare_dram_parameter("idxm", [P, m_cols], I32,
                                             isOutput=False)
    out_ext = nc.declare_dram_parameter("out", [blocks * P, D], F32, isOutput=True)

    groups = [list(range(g, min(g + BLOCKS_PER_GATHER, blocks)))
              for g in range(0, blocks, BLOCKS_PER_GATHER)]

    with tile.TileContext(nc) as tc:
        with (
            tc.tile_pool(name="dram", bufs=1, space="DRAM") as dram,
            tc.tile_pool(name="persist", bufs=1) as persist,
            tc.tile_pool(name="xin", bufs=3) as xin_pool,
            tc.tile_pool(name="pk", bufs=3) as pk_pool,
            tc.tile_pool(name="pscr", bufs=2) as pscr_pool,
            tc.tile_pool(name="gath", bufs=4) as gath_pool,
            tc.tile_pool(name="scr", bufs=3) as scr_pool,
            tc.tile_pool(name="sm", bufs=4) as sm_pool,
            tc.tile_pool(name="outp", bufs=3) as out_pool,
        ):
            packed = dram.tile([n_nodes_pad, DP], F32)
            packed_mini = None
            if b_mini:
                packed_mini = dram.tile([mini_rows, DP], F32, tag="pkmini")

            # ---- persistent small tiles -------------------------------------
            beta_sb = persist.tile([P, 1], F32)
            nc.sync.dma_start(out=beta_sb[:], in_=beta_ext[:, :])
            idx_sb = persist.tile([P, totk], I32)
            nc.sync.dma_start(out=idx_sb[:], in_=idx_ext[:, :])
            deg_sb = persist.tile([P, blocks], F32)
            nc.sync.dma_start(out=deg_sb[:], in_=deg_ext[:, :])
            iota_i = persist.tile([P, kmax], I32)
            nc.gpsimd.iota(iota_i[:], pattern=[[1, kmax]], base=0,
                           channel_multiplier=0)
            iota_f = persist.tile([P, kmax], F32)
            nc.vector.tensor_copy(iota_f[:], iota_i[:])
            eps_sb = persist.tile([P, 1], F32)
            nc.vector.memset(eps_sb[:], EPS)

            # ---- prep: packed rows [x_norm | w] -----------------------------
            A = prep_rows

            def prep_loop(src_ext, dst_tile, n_tiles):
                x_r = src_ext[:, :].rearrange("(s p a) d -> s p a d",
                                              p=P, a=prep_rows)
                pk_r = dst_tile[:].rearrange("(s p a) e -> s p a e",
                                             p=P, a=prep_rows)
                for s in range(n_tiles):
                    xt = xin_pool.tile([P, A, D], F32)
                    nc.sync.dma_start(out=xt[:], in_=x_r[s])
                    sq = pscr_pool.tile([P, A, D], F32)
                    nc.vector.tensor_tensor(out=sq[:], in0=xt[:], in1=xt[:],
                                            op=mybir.AluOpType.mult)
                    ss = sm_pool.tile([P, A], F32, tag="prep_ss")
                    nc.vector.tensor_reduce(out=ss[:], in_=sq[:],
                                            axis=mybir.AxisListType.X,
                                            op=mybir.AluOpType.add)
                    w = sm_pool.tile([P, A], F32, tag="prep_w")
                    nc.scalar.activation(w[:], ss[:],
                                         mybir.ActivationFunctionType.Sqrt,
                                         bias=eps_sb[:, :1])
                    winv = sm_pool.tile([P, A], F32, tag="prep_winv")
                    nc.vector.reciprocal(winv[:], w[:])
                    pk = pk_pool.tile([P, A, DP], F32)
                    nc.vector.tensor_tensor(
                        out=pk[:, :, 0:D], in0=xt[:],
                        in1=winv[:, :, None].to_broadcast([P, A, D]),
                        op=mybir.AluOpType.mult)
                    nc.vector.tensor_copy(pk[:, :, D], w[:])
                    nc.vector.memset(pk[:, :, D + 1:DP], 0.0)
                    nc.sync.dma_start(out=pk_r[s], in_=pk[:])

            if b_mini:
                idxm_sb = persist.tile([P, m_cols], I32)
                nc.sync.dma_start(out=idxm_sb[:], in_=idxm_ext[:, :])
                prep_loop(xm_ext, packed_mini, mini_rows // (P * prep_rows))
            prep_loop(x_ext, packed, supertiles)

            # ---- normalize the block-node (dst) features --------------------
            # xd is the core's own nodes' raw features in block layout
            # [p, b, d] (host shard); normalize on-device.
            xnd_all = persist.tile([P, blocks, D], F32)
            xd_sb = persist.tile([P, blocks, D], F32)
            nc.sync.dma_start(out=xd_sb[:], in_=xd_ext[:, :])
            dsq = pscr_pool.tile([P, blocks, D], F32, tag="dsq")
            nc.vector.tensor_tensor(out=dsq[:], in0=xd_sb[:], in1=xd_sb[:],
                                    op=mybir.AluOpType.mult)
            dss = persist.tile([P, blocks], F32)
            nc.vector.tensor_reduce(out=dss[:], in_=dsq[:],
                                    axis=mybir.AxisListType.X,
                                    op=mybir.AluOpType.add)
            dw = persist.tile([P, blocks], F32)
            nc.scalar.activation(dw[:], dss[:],
                                 mybir.ActivationFunctionType.Sqrt,
                                 bias=eps_sb[:, :1])
            dwinv = persist.tile([P, blocks], F32)
            nc.vector.reciprocal(dwinv[:], dw[:])
            nc.vector.tensor_tensor(
                out=xnd_all[:], in0=xd_sb[:],
                in1=dwinv[:, :, None].to_broadcast([P, blocks, D]),
                op=mybir.AluOpType.mult)

            out_r = out_ext[:, :].rearrange("(b p) d -> b p d", p=P)

            # ---- main loop --------------------------------------------------
            for grp in groups:
                g0, g1 = grp[0], grp[-1]
                o0, o1 = int(offs[g0]), int(offs[g1 + 1])
                tk = o1 - o0

                use_mini = b_mini and g1 < b_mini
                src_tile = packed_mini if use_mini else packed
                src_idx = idxm_sb if use_mini else idx_sb
                gt = gath_pool.tile([P, tk, DP], F32, tag="gath")
                for c in range(tk):
                    nc.gpsimd.indirect_dma_start(
                        out=gt[:, c, :], out_offset=None,
                        in_=src_tile[:],
                        in_offset=bass.IndirectOffsetOnAxis(
                            ap=src_idx[:, o0 + c:o0 + c + 1], axis=0),
                    )

                for b in grp:
                    kb = int(K[b])
                    oo = int(offs[b]) - o0
                    xs = gt[:, oo:oo + kb, 0:D]            # [P, kb, D]
                    wsrc = gt[:, oo:oo + kb, D]            # [P, kb]
                    xnd = xnd_all[:, b:b + 1, :]           # [P, 1, D]

                    t = scr_pool.tile([P, kmax, D], F32, tag="tscr")
                    nc.vector.tensor_tensor(
                        out=t[:, :kb, :], in0=xs,
                        in1=xnd.to_broadcast([P, kb, D]),
                        op=mybir.AluOpType.mult)
                    d0 = sm_pool.tile([P, kmax], F32, tag="d0")
                    nc.vector.tensor_reduce(out=d0[:, :kb], in_=t[:, :kb, :],
                                            axis=mybir.AxisListType.X,
                                            op=mybir.AluOpType.add)
                    z = sm_pool.tile([P, kmax], F32, tag="z")
                    nc.scalar.activation(z[:, :kb], d0[:, :kb],
                                         mybir.ActivationFunctionType.Exp,
                                         scale=beta_sb[:, :])
                    # mask pad slots: z *= (deg > k)
                    v = sm_pool.tile([P, kmax], F32, tag="v")
                    nc.vector.tensor_tensor(
                        out=v[:, :kb],
                        in0=deg_sb[:, b:b + 1].to_broadcast([P, kb]),
                        in1=iota_f[:, :kb],
                        op=mybir.AluOpType.is_gt)
                    nc.vector.tensor_tensor(out=z[:, :kb], in0=z[:, :kb],
                                            in1=v[:, :kb],
                                            op=mybir.AluOpType.mult)
                    seg = sm_pool.tile([P, 1], F32, tag="seg")
                    nc.vector.tensor_reduce(out=seg[:], in_=z[:, :kb],
                                            axis=mybir.AxisListType.X,
                                            op=mybir.AluOpType.add)
                    nc.vector.tensor_scalar_add(seg[:], seg[:], EPS)
                    rec = sm_pool.tile([P, 1], F32, tag="rec")
                    nc.vector.reciprocal(rec[:], seg[:])
                    # z' = z * w_src ; payload = z' * x_norm_src
                    nc.vector.tensor_tensor(out=z[:, :kb], in0=z[:, :kb],
                                            in1=wsrc,
                                            op=mybir.AluOpType.mult)
                    nc.vector.tensor_tensor(
                        out=t[:, :kb, :], in0=xs,
                        in1=z[:, :kb, None].to_broadcast([P, kb, D]),
                        op=mybir.AluOpType.mult)
                    ob = out_pool.tile([P, D], F32, tag="ob")
                    nc.vector.tensor_reduce(
                        out=ob[:], in_=t[:, :kb, :].rearrange("p k d -> p d k"),
                        axis=mybir.AxisListType.X,
                        op=mybir.AluOpType.add)
                    nc.vector.tensor_scalar_mul(ob[:], ob[:], rec[:, :1])
                    nc.sync.dma_start(out=out_r[b], in_=ob[:])

    return nc


# ----------------------------------------------------------------------------
# Public entry point
# ----------------------------------------------------------------------------

def _pad_rows(n_rows: int, quantum: int) -> int:
    return ((n_rows + quantum - 1) // quantum) * quantum


def kernel(x: np.ndarray, beta: np.ndarray, edge_index: np.ndarray,
           _debug_sim: bool = False) -> np.ndarray:
    x = np.asarray(x, dtype=np.float32)
    beta = np.asarray(beta, dtype=np.float32)
    edge_index = np.asarray(edge_index)
    n_nodes, d_feat = x.shape

    plan = build_plan(edge_index, n_nodes)
    blocks, slots = plan["blocks"], plan["slots"]

    n_pad = _pad_rows(n_nodes + 1, P * PREP_ROWS)
    x_pad = np.zeros((n_pad, d_feat), dtype=np.float32)
    x_pad[:n_nodes] = x

    # Mini head-start table: the first b_mini blocks gather from a small
    # dedicated table so the Pool engine starts ~120us before full prep ends.
    b_mini = 6 if blocks >= 20 else (2 if blocks >= 6 else 0)
    src64 = np.asarray(edge_index[0], dtype=np.int64)
    dst64 = np.asarray(edge_index[1], dtype=np.int64)
    plan2 = _build_core_arrays(plan, src64, dst64, n_nodes)
    mini_rows = 0
    xm = idxm = None
    if b_mini:
        offs = plan["offs"]
        m_cols = int(offs[b_mini])
        uniq = [np.unique(plan2["idx_all"][c][:, :m_cols])
                for c in range(N_CORES)]
        mini_rows = _pad_rows(max(len(u) for u in uniq) + 1, P * PREP_ROWS)
        xm = np.zeros((N_CORES, mini_rows, d_feat), dtype=np.float32)
        idxm = np.zeros((N_CORES, P, m_cols), dtype=np.int32)
        for c in range(N_CORES):
            ids = uniq[c]
            xm[c, :len(ids)] = x_pad[ids]
            idxm[c] = np.searchsorted(
                ids, plan2["idx_all"][c][:, :m_cols]).astype(np.int32)

    nc = build_kernel(n_pad, d_feat, blocks, plan["K"], plan["offs"],
                      plan["totk"], b_mini=b_mini, mini_rows=mini_rows)
    if not nc.is_finalized():
        nc.finalize()

    # per-core input maps
    in_maps = []
    beta_b = np.broadcast_to(beta.reshape(1, 1), (P, 1)).astype(np.float32).copy()
    for c in range(N_CORES):
        # core's own (dst) node features in block layout [p, b, d] — a shard
        xd = x_pad[plan2["perm"][c]].reshape(P, blocks * d_feat)
        m = {
            "x": x_pad,
            "idx": plan2["idx_all"][c],
            "xd": np.ascontiguousarray(xd),
            "deg": plan2["degm"][c],
            "beta": beta_b,
        }
        if b_mini:
            m["xm"] = xm[c]
            m["idxm"] = idxm[c]
        in_maps.append(m)

    if _debug_sim:
        from concourse import bass_interp
        sim = bass_interp.MultiCoreSim(nc, N_CORES)
        for c in range(N_CORES):
            for k, vv in in_maps[c].items():
                sim.cores[c].tensor(k)[:] = vv
        sim.simulate()
        results = [{"out": sim.cores[c].mem_tensor("out").copy()}
                   for c in range(N_CORES)]
    else:
        from concourse.bass_utils import run_bass_kernel_spmd
        trace = bool(int(os.environ.get("AGNN_TRACE", "0")))
        tmpdir = os.environ.get("AGNN_TRACE_DIR") or None
        res = run_bass_kernel_spmd(nc, in_maps, core_ids=list(range(N_CORES)),
                                   trace=trace, tmpdir=tmpdir)
        results = res.results
        global LAST_RESULT
        LAST_RESULT = res

    out = np.zeros((n_nodes, d_feat), dtype=np.float32)
    node_of = plan["node_of"]
    for c in range(N_CORES):
        nd = node_of[c]
        valid = nd >= 0
        out[nd[valid]] = results[c]["out"][:slots][valid]
    return out


def _build_core_arrays(plan, src, dst, n_nodes):
    """Recompute the per-core index arrays (kept out of plan for clarity)."""
    deg = np.bincount(dst, minlength=n_nodes).astype(np.int64)
    eorder = np.argsort(dst, kind="stable")
    src_sorted = src[eorder]
    starts = np.zeros(n_nodes + 1, dtype=np.int64)
    np.cumsum(deg, out=starts[1:])

    blocks, totk = plan["blocks"], plan["totk"]
    K, offs, dummy = plan["K"], plan["offs"], plan["dummy"]
    node_of = plan["node_of"]

    idx_all = np.full((N_CORES, P, totk), dummy, dtype=np.int32)
    perm = np.full((N_CORES, P, blocks), dummy, dtype=np.int32)
    degm = np.zeros((N_CORES, P, blocks), dtype=np.float32)
    for c in range(N_CORES):
        for b in range(blocks):
            kb = int(K[b])
            nd = node_of[c, b * P:(b + 1) * P]
            valid = nd >= 0
            ndv = np.where(valid, nd, 0)
            d = np.where(valid, deg[ndv], 0)
            perm[c, :, b] = np.where(valid, nd, dummy)
            degm[c, :, b] = d.astype(np.float32)
            kk = np.arange(kb)[None, :]
            take = kk < d[:, None]
            pos = np.where(take, starts[ndv][:, None] + kk, 0)
            idx_all[c, :, offs[b]:offs[b] + kb] = np.where(
                take, src_sorted[pos], dummy)
    return dict(idx_all=idx_all, perm=perm, degm=degm)



# revision 8
# speedup vs baseline: 6.3862x; 6.3862x over previous
"""AGNN propagation kernel for 8 TRN2 NeuronCores.

Algorithm (matches reference):
    x_norm = x * rsqrt(sum(x^2, -1) + 1e-8)
    logit_e = beta * <x_norm[dst_e], x_norm[src_e]>        (in [-beta, beta])
    alpha_e = exp(logit_e) / (segsum_dst(exp(logit)) + 1e-8)
    out_i   = sum_{e: dst_e = i} alpha_e * x[src_e]

Because |logit| <= beta < 1, the segment-max subtraction in the reference is
numerically unnecessary (exp stays in [e^-1, e]); plain exp matches to ~1e-7.

Sharding: node-parallel, no collectives. Host sorts nodes by in-degree and
stripes them across the 8 cores (rank c, c+8, ...), so every core sees an
identical degree profile. Each core packs its nodes into blocks of 128
(1 node per SBUF partition); block b is padded to K_b = max degree in the
block (tight, because nodes are degree-sorted).

Per the sharding hint, the host also gathers the raw source features per
edge slot (pure index manipulation -- no arithmetic): xe[p, slot] =
[x[src] (32) | bias | 0] where bias is -1e20 for pad slots and 0 otherwise.
The device streams these tables SEQUENTIALLY (no indirect DMA, whose
128-descriptor-per-instruction SWDGE cost dominated previous versions) and
does all the math per edge: dot with the normalized dst vector (the bias
column rides along and sends pad logits to -inf), source-norm rsqrt,
exp via the scalar engine with fused segment-sum, weighted aggregation.

Device phases (per core, one SPMD graph):
  main: per group of blocks: sequential DMA of the edge payload tile,
        fused scalar_tensor_tensor dot + square, two free-dim reduces,
        exp+segsum on the scalar engine, weighted aggregation, output rows.
Host reassembles: out[node_order] = dense rows.
"""

import os
import numpy as np

import concourse.bass as bass
import concourse.bacc as bacc
import concourse.mybir as mybir
import concourse.tile as tile

F32 = mybir.dt.float32
I32 = mybir.dt.int32

N_CORES = 8
LAST_RESULT = None  # set by kernel() for profiling harnesses
P = 128          # SBUF partitions (= nodes per block)
D = 32           # feature dim
DW = 34          # payload row: x_src(32) + bias(1) + zero(1)
EPS = 1e-8
NEG_BIAS = -1e20  # pad-slot logit bias
CLAMP = -80.0    # lower clamp on beta*cos before exp (kills pads safely)
NB = 4           # blocks per DMA group

MULT = mybir.AluOpType.mult
ADD = mybir.AluOpType.add
MAXOP = mybir.AluOpType.max


# ----------------------------------------------------------------------------
# Host-side planning (index manipulation only; no FLOPs on tensor data)
# ----------------------------------------------------------------------------

def build_plan(edge_index: np.ndarray, n_nodes: int):
    src = np.asarray(edge_index[0], dtype=np.int64)
    dst = np.asarray(edge_index[1], dtype=np.int64)

    deg = np.bincount(dst, minlength=n_nodes).astype(np.int64)

    # Global degree-descending node order, striped over cores.
    order = np.argsort(-deg, kind="stable")

    nodes_per_core = (n_nodes + N_CORES - 1) // N_CORES
    blocks = (nodes_per_core + P - 1) // P
    slots = blocks * P                      # padded nodes per core

    # CSR of incoming edges (sorted by dst).
    eorder = np.argsort(dst, kind="stable")
    src_sorted = src[eorder]
    starts = np.zeros(n_nodes + 1, dtype=np.int64)
    np.cumsum(deg, out=starts[1:])

    # Shared per-block K: block b holds global ranks [b*P*NC, (b+1)*P*NC).
    deg_ranked = deg[order]
    K = np.zeros(blocks, dtype=np.int64)
    for b in range(blocks):
        lo = b * P * N_CORES
        hi = min(lo + P * N_CORES, n_nodes)
        K[b] = max(1, int(deg_ranked[lo:hi].max()) if hi > lo else 1)

    groups = [list(range(g, min(g + NB, blocks))) for g in range(0, blocks, NB)]
    offs = np.zeros(blocks + 1, dtype=np.int64)
    np.cumsum(K, out=offs[1:])
    totk = int(offs[-1])

    dummy = n_nodes  # pad rows reference this all-zero row

    # node_of[c, s]: global node id at core c, slot s (or -1 pad).
    ranks = np.arange(slots) * N_CORES  # slot -> global rank base
    node_of = np.full((N_CORES, slots), -1, dtype=np.int64)
    for c in range(N_CORES):
        r = ranks + c
        valid = r < n_nodes
        node_of[c, valid] = order[r[valid]]

    # Per-core gather index table [P, totk] and dst permutation [P, blocks].
    idx_all = np.full((N_CORES, P, totk), dummy, dtype=np.int64)
    perm = np.full((N_CORES, P, blocks), dummy, dtype=np.int64)
    for c in range(N_CORES):
        for b in range(blocks):
            kb = int(K[b])
            kk = np.arange(kb)[None, :]
            nd = node_of[c, b * P:(b + 1) * P]
            valid = nd >= 0
            ndv = np.where(valid, nd, 0)
            d_ = np.where(valid, deg[ndv], 0)
            perm[c, :, b] = np.where(valid, nd, dummy)
            take = kk < d_[:, None]
            p_ = np.where(take, starts[ndv][:, None] + kk, 0)
            idx_all[c, :, offs[b]:offs[b] + kb] = np.where(
                take, src_sorted[p_], dummy)

    return dict(
        n_nodes=n_nodes, blocks=blocks, slots=slots, groups=groups,
        K=K, offs=offs, totk=totk, dummy=dummy, node_of=node_of,
        idx_all=idx_all, perm=perm,
    )


# ----------------------------------------------------------------------------
# Bass graph builder (one SPMD graph shared by all cores)
# ----------------------------------------------------------------------------

def build_kernel(blocks: int, groups, K, offs, totk: int):
    nc = bacc.Bacc(None, target_bir_lowering=False, debug=False)

    xe_ext = nc.declare_dram_parameter("xe", [P, totk * DW], F32, isOutput=False)
    xd_ext = nc.declare_dram_parameter("xd", [P, blocks * D], F32, isOutput=False)
    beta_ext = nc.declare_dram_parameter("beta", [P, 1], F32, isOutput=False)
    out_ext = nc.declare_dram_parameter("out", [blocks * P, D], F32, isOutput=True)

    with tile.TileContext(nc) as tc:
        with (
            tc.tile_pool(name="persist", bufs=1) as persist,
            tc.tile_pool(name="xin", bufs=3) as xin_pool,
            tc.tile_pool(name="scr", bufs=2) as scr_pool,
            tc.tile_pool(name="t2p", bufs=2) as t2_pool,
            tc.tile_pool(name="sm", bufs=3) as sm_pool,
            tc.tile_pool(name="outp", bufs=3) as out_pool,
        ):
            # ---- persistent small tiles -------------------------------------
            beta_sb = persist.tile([P, 1], F32)
            nc.sync.dma_start(out=beta_sb[:], in_=beta_ext[:, :])
            eps_sb = persist.tile([P, 1], F32)
            nc.vector.memset(eps_sb[:], EPS)

            # ---- normalize the block-node (dst) features --------------------
            # xd is the core's own nodes' raw features in block layout
            # [p, b, d] (host shard); normalize on-device, with a trailing
            # [1.0, 0.0] so the 34-wide dot picks up the bias field.
            xd_sb = persist.tile([P, blocks, D], F32)
            nc.sync.dma_start(out=xd_sb[:], in_=xd_ext[:, :])
            dsq = scr_pool.tile([P, blocks, D], F32, tag="dsq")
            nc.vector.scalar_tensor_tensor(
                out=dsq[:], in0=xd_sb[:], scalar=1.0, in1=xd_sb[:],
                op0=MULT, op1=MULT)
            dss = persist.tile([P, blocks], F32)
            nc.vector.tensor_reduce(out=dss[:], in_=dsq[:],
                                    axis=mybir.AxisListType.X, op=ADD)
            dw = persist.tile([P, blocks], F32)
            nc.scalar.activation(dw[:], dss[:],
                                 mybir.ActivationFunctionType.Sqrt,
                                 bias=eps_sb[:, :1])
            dwinv = persist.tile([P, blocks], F32)
            nc.vector.reciprocal(dwinv[:], dw[:])
            xnd_all = persist.tile([P, blocks, DW], F32)
            nc.vector.scalar_tensor_tensor(
                out=xnd_all[:, :, 0:D], in0=xd_sb[:], scalar=1.0,
                in1=dwinv[:, :, None].to_broadcast([P, blocks, D]),
                op0=MULT, op1=MULT)
            nc.vector.memset(xnd_all[:, :, D:D + 1], 1.0)
            nc.vector.memset(xnd_all[:, :, D + 1:DW], 0.0)

            out_r = out_ext[:, :].rearrange("(b p) d -> b p d", p=P)
            xe_r = xe_ext[:, :].rearrange("p (s e) -> p s e", e=DW)

            # ---- main loop --------------------------------------------------
            for gi, grp in enumerate(groups):
                g0 = grp[0]
                nb = len(grp)
                o0, o1 = int(offs[g0]), int(offs[grp[-1] + 1])
                tk = o1 - o0

                xeg = xin_pool.tile([P, tk, DW], F32, tag="xeg")
                nc.sync.dma_start(out=xeg[:], in_=xe_r[:, o0:o1, :])

                # t = xe * xnd (34 wide; bias col rides along), per block
                t = scr_pool.tile([P, tk, DW], F32, tag="t")
                for j, b in enumerate(grp):
                    js = slice(int(offs[b]) - o0, int(offs[b + 1]) - o0)
                    nc.vector.scalar_tensor_tensor(
                        out=t[:, js, :], in0=xeg[:, js, :], scalar=1.0,
                        in1=xnd_all[:, b, None, :].to_broadcast(
                            [P, js.stop - js.start, DW]),
                        op0=MULT, op1=MULT)
                d0 = sm_pool.tile([P, tk], F32, tag="d0")
                nc.vector.tensor_reduce(out=d0[:], in_=t[:],
                                        axis=mybir.AxisListType.X, op=ADD)

                # ss = sum(xe^2) (reuses t's buffer after d0 consumed it)
                nc.vector.scalar_tensor_tensor(
                    out=t[:, :, 0:D], in0=xeg[:, :, 0:D], scalar=1.0,
                    in1=xeg[:, :, 0:D], op0=MULT, op1=MULT)
                ss = sm_pool.tile([P, tk], F32, tag="ss")
                nc.vector.tensor_reduce(out=ss[:], in_=t[:, :, 0:D],
                                        axis=mybir.AxisListType.X, op=ADD)
                w = sm_pool.tile([P, tk], F32, tag="w")
                nc.scalar.activation(w[:], ss[:],
                                     mybir.ActivationFunctionType.Sqrt,
                                     bias=eps_sb[:, :1])
                winv = sm_pool.tile([P, tk], F32, tag="winv")
                nc.vector.reciprocal(winv[:], w[:])

                # logit = clamp(beta * d0 * winv, CLAMP); pads -> -inf-ish
                cosb = sm_pool.tile([P, tk], F32, tag="cosb")
                nc.vector.scalar_tensor_tensor(
                    out=cosb[:], in0=d0[:], scalar=beta_sb[:, :1],
                    in1=winv[:], op0=MULT, op1=MULT)
                nc.vector.tensor_scalar_max(cosb[:], cosb[:], CLAMP)

                # per-block exp + fused segment-sum on the scalar engine
                z = sm_pool.tile([P, tk], F32, tag="z")
                seg = sm_pool.tile([P, nb], F32, tag="seg")
                for j, b in enumerate(grp):
                    js = slice(int(offs[b]) - o0, int(offs[b + 1]) - o0)
                    nc.scalar.activation(
                        z[:, js], cosb[:, js],
                        mybir.ActivationFunctionType.Exp,
                        accum_out=seg[:, j:j + 1])

                # t2[p, d, k] = xe[p, k, d] * z[p, k]; reduce over k
                ov = out_pool.tile([P, nb, D], F32, tag="ov")
                kmax = max(int(K[b]) for b in grp)
                for j, b in enumerate(grp):
                    js = slice(int(offs[b]) - o0, int(offs[b + 1]) - o0)
                    kb = js.stop - js.start
                    t2f = t2_pool.tile([P, D, kmax], F32, tag="t2")
                    t2 = t2f[:, :, 0:kb]
                    nc.vector.scalar_tensor_tensor(
                        out=t2.rearrange("p d k -> p k d"),
                        in0=xeg[:, js, 0:D], scalar=1.0,
                        in1=z[:, js, None].to_broadcast([P, kb, D]),
                        op0=MULT, op1=MULT)
                    nc.vector.tensor_reduce(out=ov[:, j, :], in_=t2,
                                            axis=mybir.AxisListType.X, op=ADD)

                # ov /= (seg + eps)
                nc.vector.tensor_scalar_add(seg[:], seg[:], EPS)
                rec = sm_pool.tile([P, nb], F32, tag="rec")
                nc.vector.reciprocal(rec[:], seg[:])
                ob = out_pool.tile([P, nb, D], F32, tag="ob")
                nc.vector.scalar_tensor_tensor(
                    out=ob[:], in0=ov[:], scalar=1.0,
                    in1=rec[:, :, None].to_broadcast([P, nb, D]),
                    op0=MULT, op1=MULT)
                for j, b in enumerate(grp):
                    nc.sync.dma_start(out=out_r[b], in_=ob[:, j, :])

    return nc


# ----------------------------------------------------------------------------
# Public entry point
# ----------------------------------------------------------------------------

def kernel(x: np.ndarray, beta: np.ndarray, edge_index: np.ndarray,
           _debug_sim: bool = False) -> np.ndarray:
    x = np.asarray(x, dtype=np.float32)
    beta = np.asarray(beta, dtype=np.float32)
    edge_index = np.asarray(edge_index)
    n_nodes, d_feat = x.shape
    assert d_feat == D

    plan = build_plan(edge_index, n_nodes)
    blocks, slots, totk = plan["blocks"], plan["slots"], plan["totk"]

    # base rows: [x (32) | bias | 0]; the dummy row is zero with bias -1e20
    base = np.zeros((n_nodes + 1, DW), dtype=np.float32)
    base[:n_nodes, 0:D] = x
    base[n_nodes, D] = NEG_BIAS

    nc = build_kernel(blocks, plan["groups"], plan["K"], plan["offs"], totk)
    if not nc.is_finalized():
        nc.finalize()

    # per-core input maps (host gather = index manipulation only)
    in_maps = []
    beta_b = np.broadcast_to(beta.reshape(1, 1), (P, 1)).astype(np.float32).copy()
    for c in range(N_CORES):
        xe = base[plan["idx_all"][c]]              # [P, totk, DW]
        xd = base[plan["perm"][c], 0:D]            # [P, blocks, D]
        in_maps.append({
            "xe": np.ascontiguousarray(xe.reshape(P, totk * DW)),
            "xd": np.ascontiguousarray(xd.reshape(P, blocks * D)),
            "beta": beta_b,
        })

    if _debug_sim:
        from concourse import bass_interp
        sim = bass_interp.MultiCoreSim(nc, N_CORES)
        for c in range(N_CORES):
            for k, vv in in_maps[c].items():
                sim.cores[c].tensor(k)[:] = vv
        sim.simulate()
        results = [{"out": sim.cores[c].mem_tensor("out").copy()}
                   for c in range(N_CORES)]
    else:
        from concourse.bass_utils import run_bass_kernel_spmd
        trace = bool(int(os.environ.get("AGNN_TRACE", "0")))
        tmpdir = os.environ.get("AGNN_TRACE_DIR") or None
        res = run_bass_kernel_spmd(nc, in_maps, core_ids=list(range(N_CORES)),
                                   trace=trace, tmpdir=tmpdir)
        results = res.results
        global LAST_RESULT
        LAST_RESULT = res

    out = np.zeros((n_nodes, d_feat), dtype=np.float32)
    node_of = plan["node_of"]
    for c in range(N_CORES):
        nd = node_of[c]
        valid = nd >= 0
        out[nd[valid]] = results[c]["out"][:slots][valid]
    return out


# revision 13
# speedup vs baseline: 7.4196x; 1.1618x over previous
"""AGNN propagation kernel for 8 TRN2 NeuronCores.

Algorithm (matches reference):
    x_norm = x * rsqrt(sum(x^2, -1) + 1e-8)
    logit_e = beta * <x_norm[dst_e], x_norm[src_e]>        (in [-beta, beta])
    alpha_e = exp(logit_e) / (segsum_dst(exp(logit)) + 1e-8)
    out_i   = sum_{e: dst_e = i} alpha_e * x[src_e]

Because |logit| <= beta < 1, the segment-max subtraction in the reference is
numerically unnecessary (exp stays in [e^-1, e]); plain exp matches to ~1e-7.

Sharding: node-parallel, no collectives. Host sorts nodes by in-degree and
stripes them across the 8 cores (rank c, c+8, ...), so every core sees an
identical degree profile. Each core packs its nodes into blocks of 128
(1 node per SBUF partition); block b is padded to K_b = max degree in the
block (tight, because nodes are degree-sorted).

Per the sharding hint, the host also gathers the raw source features per
edge slot (pure index manipulation -- no arithmetic): xe[p, slot] =
[x[src] (32) | bias | 0] where bias is -1e20 for pad slots and 0 otherwise.
The device streams these tables SEQUENTIALLY (no indirect DMA, whose
128-descriptor-per-instruction SWDGE cost dominated previous versions) and
does all the math per edge: dot with the normalized dst vector (the bias
column rides along and sends pad logits to -inf), source-norm rsqrt,
exp via the scalar engine with fused segment-sum, weighted aggregation.

Device phases (per core, one SPMD graph):
  main: per group of blocks: sequential DMA of the edge payload tile,
        fused scalar_tensor_tensor dot + square, two free-dim reduces,
        exp+segsum on the scalar engine, weighted aggregation, output rows.
Host reassembles: out[node_order] = dense rows.
"""

import os
import numpy as np

import concourse.bass as bass
import concourse.bacc as bacc
import concourse.mybir as mybir
import concourse.tile as tile

F32 = mybir.dt.float32
I32 = mybir.dt.int32

N_CORES = 8
LAST_RESULT = None  # set by kernel() for profiling harnesses
P = 128          # SBUF partitions (= nodes per block)
D = 32           # feature dim
DW = 34          # payload row: x_src(32) + bias(1) + zero(1)
EPS = 1e-8
NEG_BIAS = -1e20  # pad-slot logit bias
CLAMP = -80.0    # lower clamp on beta*cos before exp (kills pads safely)
NB = 4           # blocks per DMA group

MULT = mybir.AluOpType.mult
ADD = mybir.AluOpType.add
MAXOP = mybir.AluOpType.max


# ----------------------------------------------------------------------------
# Host-side planning (index manipulation only; no FLOPs on tensor data)
# ----------------------------------------------------------------------------

def build_plan(edge_index: np.ndarray, n_nodes: int):
    src = np.asarray(edge_index[0], dtype=np.int64)
    dst = np.asarray(edge_index[1], dtype=np.int64)

    deg = np.bincount(dst, minlength=n_nodes).astype(np.int64)

    # Global degree-descending node order, striped over cores.
    order = np.argsort(-deg, kind="stable")

    nodes_per_core = (n_nodes + N_CORES - 1) // N_CORES
    blocks = (nodes_per_core + P - 1) // P
    slots = blocks * P                      # padded nodes per core

    # CSR of incoming edges (sorted by dst).
    eorder = np.argsort(dst, kind="stable")
    src_sorted = src[eorder]
    starts = np.zeros(n_nodes + 1, dtype=np.int64)
    np.cumsum(deg, out=starts[1:])

    # Shared per-block K: block b holds global ranks [b*P*NC, (b+1)*P*NC).
    deg_ranked = deg[order]
    K = np.zeros(blocks, dtype=np.int64)
    for b in range(blocks):
        lo = b * P * N_CORES
        hi = min(lo + P * N_CORES, n_nodes)
        K[b] = max(1, int(deg_ranked[lo:hi].max()) if hi > lo else 1)

    groups = [list(range(g, min(g + NB, blocks))) for g in range(0, blocks, NB)]
    offs = np.zeros(blocks + 1, dtype=np.int64)
    np.cumsum(K, out=offs[1:])
    totk = int(offs[-1])

    dummy = n_nodes  # pad rows reference this all-zero row

    # node_of[c, s]: global node id at core c, slot s (or -1 pad).
    ranks = np.arange(slots) * N_CORES  # slot -> global rank base
    node_of = np.full((N_CORES, slots), -1, dtype=np.int64)
    for c in range(N_CORES):
        r = ranks + c
        valid = r < n_nodes
        node_of[c, valid] = order[r[valid]]

    # Per-core gather index table [P, totk] and dst permutation [P, blocks].
    idx_all = np.full((N_CORES, P, totk), dummy, dtype=np.int64)
    perm = np.full((N_CORES, P, blocks), dummy, dtype=np.int64)
    for c in range(N_CORES):
        for b in range(blocks):
            kb = int(K[b])
            kk = np.arange(kb)[None, :]
            nd = node_of[c, b * P:(b + 1) * P]
            valid = nd >= 0
            ndv = np.where(valid, nd, 0)
            d_ = np.where(valid, deg[ndv], 0)
            perm[c, :, b] = np.where(valid, nd, dummy)
            take = kk < d_[:, None]
            p_ = np.where(take, starts[ndv][:, None] + kk, 0)
            idx_all[c, :, offs[b]:offs[b] + kb] = np.where(
                take, src_sorted[p_], dummy)

    return dict(
        n_nodes=n_nodes, blocks=blocks, slots=slots, groups=groups,
        K=K, offs=offs, totk=totk, dummy=dummy, node_of=node_of,
        idx_all=idx_all, perm=perm,
    )


# ----------------------------------------------------------------------------
# Bass graph builder (one SPMD graph shared by all cores)
# ----------------------------------------------------------------------------

def build_kernel(blocks: int, groups, K, offs, totk: int):
    nc = bacc.Bacc(None, target_bir_lowering=False, debug=False)

    xe_ext = nc.declare_dram_parameter("xe", [P, totk * DW], F32, isOutput=False)
    xd_ext = nc.declare_dram_parameter("xd", [P, blocks * D], F32, isOutput=False)
    beta_ext = nc.declare_dram_parameter("beta", [P, 1], F32, isOutput=False)
    out_ext = nc.declare_dram_parameter("out", [blocks * P, D], F32, isOutput=True)

    with tile.TileContext(nc) as tc:
        with (
            tc.tile_pool(name="persist", bufs=1) as persist,
            tc.tile_pool(name="xin", bufs=2) as xin_pool,
            tc.tile_pool(name="scr", bufs=2) as scr_pool,
            tc.tile_pool(name="sqp", bufs=2) as sq_pool,
            tc.tile_pool(name="t2p", bufs=2) as t2_pool,
            tc.tile_pool(name="sm", bufs=3) as sm_pool,
            tc.tile_pool(name="outp", bufs=3) as out_pool,
        ):
            # ---- persistent small tiles -------------------------------------
            beta_sb = persist.tile([P, 1], F32)
            nc.sync.dma_start(out=beta_sb[:], in_=beta_ext[:, :])
            eps_sb = persist.tile([P, 1], F32)
            nc.vector.memset(eps_sb[:], EPS)

            # ---- normalize the block-node (dst) features --------------------
            # xd is the core's own nodes' raw features in block layout
            # [p, b, d] (host shard); normalize on-device, with a trailing
            # [1.0, 0.0] so the 34-wide dot picks up the bias field.
            xd_sb = persist.tile([P, blocks, D], F32)
            nc.sync.dma_start(out=xd_sb[:], in_=xd_ext[:, :])
            dsq = scr_pool.tile([P, blocks, D], F32, tag="dsq")
            nc.vector.scalar_tensor_tensor(
                out=dsq[:], in0=xd_sb[:], scalar=1.0, in1=xd_sb[:],
                op0=MULT, op1=MULT)
            dss = persist.tile([P, blocks], F32)
            nc.vector.tensor_reduce(out=dss[:], in_=dsq[:],
                                    axis=mybir.AxisListType.X, op=ADD)
            # dwinv = rsqrt(dss + eps) = exp(-0.5 * ln(dss + eps)); Ln, Exp
            # and Square share one activation table set -> no table reloads
            dw = persist.tile([P, blocks], F32)
            nc.scalar.activation(dw[:], dss[:],
                                 mybir.ActivationFunctionType.Ln,
                                 bias=eps_sb[:, :1])
            dwinv = persist.tile([P, blocks], F32)
            nc.scalar.activation(dwinv[:], dw[:],
                                 mybir.ActivationFunctionType.Exp,
                                 scale=-0.5)
            xnd_all = persist.tile([P, blocks, DW], F32)
            nc.vector.scalar_tensor_tensor(
                out=xnd_all[:, :, 0:D], in0=xd_sb[:], scalar=1.0,
                in1=dwinv[:, :, None].to_broadcast([P, blocks, D]),
                op0=MULT, op1=MULT)
            nc.vector.memset(xnd_all[:, :, D:D + 1], 1.0)
            nc.vector.memset(xnd_all[:, :, D + 1:DW], 0.0)

            out_r = out_ext[:, :].rearrange("(b p) d -> b p d", p=P)
            xe_r = xe_ext[:, :].rearrange("p (s e) -> p s e", e=DW)

            # ---- main loop --------------------------------------------------
            for gi, grp in enumerate(groups):
                g0 = grp[0]
                nb = len(grp)
                o0, o1 = int(offs[g0]), int(offs[grp[-1] + 1])
                tk = o1 - o0

                xeg = xin_pool.tile([P, tk, DW], F32, tag="xeg")
                nc.sync.dma_start(out=xeg[:], in_=xe_r[:, o0:o1, :])

                # t = xe * xnd (34 wide; bias col rides along), per block
                t = scr_pool.tile([P, tk, DW], F32, tag="t")
                for j, b in enumerate(grp):
                    js = slice(int(offs[b]) - o0, int(offs[b + 1]) - o0)
                    nc.vector.scalar_tensor_tensor(
                        out=t[:, js, :], in0=xeg[:, js, :], scalar=1.0,
                        in1=xnd_all[:, b, None, :].to_broadcast(
                            [P, js.stop - js.start, DW]),
                        op0=MULT, op1=MULT)
                d0 = sm_pool.tile([P, tk], F32, tag="d0")
                nc.vector.tensor_reduce(out=d0[:], in_=t[:],
                                        axis=mybir.AxisListType.X, op=ADD)

                # ss = sum(xe^2): square on the scalar engine (frees DVE)
                sqt = sq_pool.tile([P, tk, D], F32, tag="sqt")
                nc.scalar.activation(sqt[:], xeg[:, :, 0:D],
                                     mybir.ActivationFunctionType.Square)
                ss = sm_pool.tile([P, tk], F32, tag="ss")
                nc.vector.tensor_reduce(out=ss[:], in_=sqt[:],
                                        axis=mybir.AxisListType.X, op=ADD)
                # winv = rsqrt(ss + eps) via exp(-0.5 * ln(ss + eps))
                lns = sm_pool.tile([P, tk], F32, tag="lns")
                nc.scalar.activation(lns[:], ss[:],
                                     mybir.ActivationFunctionType.Ln,
                                     bias=eps_sb[:, :1])
                winv = sm_pool.tile([P, tk], F32, tag="winv")
                nc.scalar.activation(winv[:], lns[:],
                                     mybir.ActivationFunctionType.Exp,
                                     scale=-0.5)

                # logit = clamp(beta * d0 * winv, CLAMP); pads -> -inf-ish
                cosb = sm_pool.tile([P, tk], F32, tag="cosb")
                nc.vector.scalar_tensor_tensor(
                    out=cosb[:], in0=d0[:], scalar=beta_sb[:, :1],
                    in1=winv[:], op0=MULT, op1=MULT)
                nc.vector.tensor_scalar_max(cosb[:], cosb[:], CLAMP)

                # per-block exp + fused segment-sum on the scalar engine
                z = sm_pool.tile([P, tk], F32, tag="z")
                seg = sm_pool.tile([P, nb], F32, tag="seg")
                for j, b in enumerate(grp):
                    js = slice(int(offs[b]) - o0, int(offs[b + 1]) - o0)
                    nc.scalar.activation(
                        z[:, js], cosb[:, js],
                        mybir.ActivationFunctionType.Exp,
                        accum_out=seg[:, j:j + 1])

                # t2[p, d, k] = xe[p, k, d] * z[p, k]; reduce over k
                ov = out_pool.tile([P, nb, D], F32, tag="ov")
                kmax = max(int(K[b]) for b in grp)
                for j, b in enumerate(grp):
                    js = slice(int(offs[b]) - o0, int(offs[b + 1]) - o0)
                    kb = js.stop - js.start
                    t2f = t2_pool.tile([P, D, kmax], F32, tag="t2")
                    t2 = t2f[:, :, 0:kb]
                    nc.vector.scalar_tensor_tensor(
                        out=t2.rearrange("p d k -> p k d"),
                        in0=xeg[:, js, 0:D], scalar=1.0,
                        in1=z[:, js, None].to_broadcast([P, kb, D]),
                        op0=MULT, op1=MULT)
                    nc.vector.tensor_reduce(out=ov[:, j, :], in_=t2,
                                            axis=mybir.AxisListType.X, op=ADD)

                # ov /= (seg + eps)
                nc.vector.tensor_scalar_add(seg[:], seg[:], EPS)
                rec = sm_pool.tile([P, nb], F32, tag="rec")
                nc.vector.reciprocal(rec[:], seg[:])
                ob = out_pool.tile([P, nb, D], F32, tag="ob")
                nc.vector.scalar_tensor_tensor(
                    out=ob[:], in0=ov[:], scalar=1.0,
                    in1=rec[:, :, None].to_broadcast([P, nb, D]),
                    op0=MULT, op1=MULT)
                for j, b in enumerate(grp):
                    nc.sync.dma_start(out=out_r[b], in_=ob[:, j, :])

    return nc


# ----------------------------------------------------------------------------
# Public entry point
# ----------------------------------------------------------------------------

def kernel(x: np.ndarray, beta: np.ndarray, edge_index: np.ndarray,
           _debug_sim: bool = False) -> np.ndarray:
    x = np.asarray(x, dtype=np.float32)
    beta = np.asarray(beta, dtype=np.float32)
    edge_index = np.asarray(edge_index)
    n_nodes, d_feat = x.shape
    assert d_feat == D

    plan = build_plan(edge_index, n_nodes)
    blocks, slots, totk = plan["blocks"], plan["slots"], plan["totk"]

    # base rows: [x (32) | bias | 0]; the dummy row is zero with bias -1e20
    base = np.zeros((n_nodes + 1, DW), dtype=np.float32)
    base[:n_nodes, 0:D] = x
    base[n_nodes, D] = NEG_BIAS

    nc = build_kernel(blocks, plan["groups"], plan["K"], plan["offs"], totk)
    if not nc.is_finalized():
        nc.finalize()

    # per-core input maps (host gather = index manipulation only)
    in_maps = []
    beta_b = np.broadcast_to(beta.reshape(1, 1), (P, 1)).astype(np.float32).copy()
    for c in range(N_CORES):
        xe = base[plan["idx_all"][c]]              # [P, totk, DW]
        xd = base[plan["perm"][c], 0:D]            # [P, blocks, D]
        in_maps.append({
            "xe": np.ascontiguousarray(xe.reshape(P, totk * DW)),
            "xd": np.ascontiguousarray(xd.reshape(P, blocks * D)),
            "beta": beta_b,
        })

    if _debug_sim:
        from concourse import bass_interp
        sim = bass_interp.MultiCoreSim(nc, N_CORES)
        for c in range(N_CORES):
            for k, vv in in_maps[c].items():
                sim.cores[c].tensor(k)[:] = vv
        sim.simulate()
        results = [{"out": sim.cores[c].mem_tensor("out").copy()}
                   for c in range(N_CORES)]
    else:
        from concourse.bass_utils import run_bass_kernel_spmd
        trace = bool(int(os.environ.get("AGNN_TRACE", "0")))
        tmpdir = os.environ.get("AGNN_TRACE_DIR") or None
        res = run_bass_kernel_spmd(nc, in_maps, core_ids=list(range(N_CORES)),
                                   trace=trace, tmpdir=tmpdir)
        results = res.results
        global LAST_RESULT
        LAST_RESULT = res

    out = np.zeros((n_nodes, d_feat), dtype=np.float32)
    node_of = plan["node_of"]
    for c in range(N_CORES):
        nd = node_of[c]
        valid = nd >= 0
        out[nd[valid]] = results[c]["out"][:slots][valid]
    return out
